# revision 2
# baseline (speedup 1.0000x reference)
"""Trainium2 Bass kernel for nn_Enhanced_transformer (dense transformer block).

v9 = v8 + ring discipline: every load (weights incl.) rides the SP ring,
interleaved one weight blob per pass-1 chunk behind the x prefetch; x
chunk loads are issued one chunk ahead in both passes; out-stores go on
SP after the next chunk's prefetch. att2f bias-add and negmu move to DVE
(fewer ACT activation-table swaps).
v8 = v7 + phase-boundary overlap fixes: att/v_w in bf16, v_w + t1_b
preloaded during pass 1, x-chunk tiles in a cross-phase pool (pass-2
prefetch starts during phase B), b'/cs round-trips on the gpsimd ring so
they never block ACT compute.
v7 = v6 + LN1 also folded into the x_q GEMM: x_q = rho_t*((x@qk) -
mu_t*colsum(qk)); the token-partition-oriented mu/rho come from PE
transposes of the broadcast stats (identity matrix shipped as an input).
No h tensor anywhere; every GEMM runs on raw f32r x.
v6 = v5 + batched DMA (one multi-level-AP descriptor per chunk / weight /
bias vector instead of per-tile triggers; a dma_start costs ~0.7us of issuing
engine sequencer time and big descriptors run at full fabric bandwidth).
v5 = v4 + LN1 folded into the t_out GEMM (no h in pass 2 at all):
  t_out = rho * (x @ W' - mu * colsum(W')) ; colsum via -1-matmul + tiny
  DRAM round-trip (same pattern as b'). Pass 2 GEMMs directly on the raw
  f32r x tiles that the residual needs anyway, so per-chunk DMA drops to
  x-in + out-out and pass 1 only streams x once (no h spill).
- mu/rho kept in SBUF (bf16) from pass 1; LN2 folded into m1 as in v3
- DMA rings split: SP = bulk loads only, ACT = weights + b'/cs + stores
See kernel_v2 docstring for the base structure (W' fusion, f32r logits, no
DRAM spills). Numpy-faithful dtype sim: L2 ~5.8e-3 (gate 2e-2).
"""

import numpy as np
import ml_dtypes

import concourse.bass as bass
import concourse.tile as tile
from concourse import bacc, mybir
from concourse import bass_utils

F32 = mybir.dt.float32
F32R = mybir.dt.float32r
BF16 = mybir.dt.bfloat16
AF = mybir.ActivationFunctionType
ALU = mybir.AluOpType
AX = mybir.AxisListType

B, N, P = 8, 4096, 1024
P4 = P // 4          # 256
EPS = 1e-5
CH = 512             # token chunk
NCH = N // CH        # 8
KP = P // 128        # 8 channel tiles
KQ = P4 // 128       # 2


def _build(qk_bias: bool, loop_R: int = 1):
    nc = bacc.Bacc("TRN2", target_bir_lowering=False, debug=False)

    xT_d = nc.dram_tensor("xT", [P, N], F32R, kind="ExternalInput").ap()
    qk_wT_d = nc.dram_tensor("qk_wT", [P, P4], F32R, kind="ExternalInput").ap()
    vw_d = nc.dram_tensor("vw", [P, P], BF16, kind="ExternalInput").ap()
    t1_wT_d = nc.dram_tensor("t1_wT", [P4, P], F32R, kind="ExternalInput").ap()
    t2_wT_d = nc.dram_tensor("t2_wT", [P4, P], F32R, kind="ExternalInput").ap()
    m1_wT_d = nc.dram_tensor("m1_wT", [P, P], BF16, kind="ExternalInput").ap()
    m2_wT_d = nc.dram_tensor("m2_wT", [P, P], BF16, kind="ExternalInput").ap()
    v_b_d = nc.dram_tensor("v_b", [P], F32, kind="ExternalInput").ap()
    t1_b_d = nc.dram_tensor("t1_b", [P], F32, kind="ExternalInput").ap()
    t2_b_d = nc.dram_tensor("t2_b", [P], F32, kind="ExternalInput").ap()
    m1_b_d = nc.dram_tensor("m1_b", [P], F32, kind="ExternalInput").ap()
    m2_b_d = nc.dram_tensor("m2_b", [P], F32, kind="ExternalInput").ap()
    m1_rs_d = nc.dram_tensor("m1_rs", [P], F32, kind="ExternalInput").ap()
    nqcs_d = nc.dram_tensor("nqcs", [P4], F32, kind="ExternalInput").ap()
    ident_d = nc.dram_tensor("ident", [128, 128], F32, kind="ExternalInput").ap()
    if qk_bias:
        qkb_d = nc.dram_tensor("qk_b", [P4], F32, kind="ExternalInput").ap()
    outT_d = nc.dram_tensor("outT", [P, N], F32, kind="ExternalOutput").ap()

    def part_bias_tiles(pool, dram_ap, name, engine=None):
        """[P] dram vector -> one [128, KP] tile, one DMA; returns slices."""
        bt = pool.tile([128, KP], F32, tag=name, name=name)
        src = bass.AP(tensor=dram_ap.tensor, offset=dram_ap.offset,
                      ap=[[1, 128], [128, KP]])
        (engine or nc.sync).dma_start(bt[:], src)
        return [bt[:, t : t + 1] for t in range(KP)]

    def blocked_dram_ap(dram_ap, row_stride, nblk, blk_rows, width, col_off=0):
        """AP iterating (q in 128, blk in nblk, j in width):
        dram[blk*blk_rows + q, col_off + j] for a row-major [rows, row_stride]
        dram tensor."""
        return bass.AP(
            tensor=dram_ap.tensor,
            offset=dram_ap.offset + col_off,
            ap=[[row_stride, 128], [row_stride * blk_rows, nblk], [1, width]],
        )

    def bcast_dma(pool, dram_ap, width, name):
        bt = pool.tile([128, width], F32, tag=name, name=name)
        src = bass.AP(
            tensor=dram_ap.tensor, offset=dram_ap.offset,
            ap=[[0, 128], *dram_ap.ap],
        )
        nc.sync.dma_start(bt[:], src)
        return bt

    with tile.TileContext(nc) as tc:
        with (
            tc.tile_pool(name="dram", bufs=1, space="DRAM") as dramp,
            tc.tile_pool(name="consts", bufs=1) as consts,
        ):
            bp_dram = dramp.tile([P], F32, name="bp_dram")
            cs_dram = dramp.tile([P], F32, name="cs_dram")

            ones_f = consts.tile([128, 128], F32, tag="ones_f", name="ones_f")
            nc.vector.memset(ones_f[:], 1.0 / P)
            ones_s = consts.tile([128, 128], F32R, tag="ones_s", name="ones_s")
            nc.vector.tensor_copy(ones_s[:], ones_f[:])
            ones_sb = consts.tile([128, 128], BF16, tag="ones_sb", name="ones_sb")
            nc.vector.tensor_copy(ones_sb[:], ones_f[:])
            ones_g = consts.tile([128, 128], F32, tag="ones_g", name="ones_g")
            nc.vector.memset(ones_g[:], 1.0)
            ones1 = consts.tile([128, 128], F32R, tag="ones1", name="ones1")
            nc.vector.tensor_copy(ones1[:], ones_g[:])
            ones_h = consts.tile([128, 128], F32, tag="ones_h", name="ones_h")
            nc.vector.memset(ones_h[:], -1.0)
            onesm1 = consts.tile([128, 128], F32R, tag="onesm1", name="onesm1")
            nc.vector.tensor_copy(onesm1[:], ones_h[:])
            eps_t = consts.tile([128, 1], F32, tag="eps", name="eps_t")
            nc.vector.memset(eps_t[:], EPS)
            ident_t = consts.tile([128, 128], F32, tag="ident", name="ident")
            nc.sync.dma_start(ident_t[:], ident_d[:, :])
            mu_st = [consts.tile([128, CH], BF16, tag=f"must{c}",
                                 name=f"must{c}") for c in range(NCH)]
            rho_st = [consts.tile([128, CH], BF16, tag=f"rhst{c}",
                                  name=f"rhst{c}") for c in range(NCH)]

            from contextlib import ExitStack as _ES
            _loop_ctx = _ES()
            if loop_R > 1:
                _loop_ctx.enter_context(tc.For_i(0, loop_R, 1))

            with (
                tc.tile_pool(name="wper", bufs=1) as wper,
                tc.tile_pool(name="px", bufs=1) as px,
            ):
                # allocate persistent weight tiles; DMAs are issued later in
                # priority order (qk -> t1/t2 -> m1/m2)
                m1_all = wper.tile([128, KP * P], BF16, tag="m1w", name="m1w")
                m1_t = [m1_all[:, p * P : (p + 1) * P] for p in range(KP)]
                m2_all = wper.tile([128, KP * P], BF16, tag="m2w", name="m2w")
                m2_t = [m2_all[:, p * P : (p + 1) * P] for p in range(KP)]
                Wp = [wper.tile([128, P], F32R, tag=f"Wp{p}", name=f"Wp{p}")
                      for p in range(KP)]

                with tc.tile_pool(name="wB12", bufs=1) as wB12:
                    t1_all = wB12.tile([128, KQ * P], F32R, tag="t1w", name="t1w")
                    t1_t = [t1_all[:, qh * P : (qh + 1) * P] for qh in range(KQ)]
                    t2_all = wB12.tile([128, KQ * P], F32R, tag="t2w", name="t2w")
                    t2_t = [t2_all[:, qh * P : (qh + 1) * P] for qh in range(KQ)]
                    vw_all = wB12.tile([128, KP * P], BF16, tag="vw", name="vw")
                    vw_t = [vw_all[:, o * P : (o + 1) * P] for o in range(KP)]

                    with tc.tile_pool(name="psE", bufs=1, space="PSUM") as psE:
                        e_ps = [psE.tile([128, P4], F32, tag=f"e{i}",
                                         name=f"e_ps{i}")[:] for i in range(KQ)]

                        # ========= PASS 1: LN1 stats, x_q, energy =========
                        with (
                            tc.tile_pool(name="p1", bufs=1) as p1,
                            tc.tile_pool(name="ps1", bufs=1, space="PSUM") as ps1,
                        ):
                            def xt_load(c):
                                t = px.tile([128, KP * CH], F32R,
                                            tag="xt", name="xt", bufs=2)
                                nc.sync.dma_start(
                                    t[:],
                                    blocked_dram_ap(xT_d, N, KP, 128, CH,
                                                    col_off=c * CH))
                                return t

                            xt_pending = {0: xt_load(0)}
                            qk_all = p1.tile([128, KP * P4], F32R, tag="qkw",
                                             name="qkw")
                            nc.sync.dma_start(
                                qk_all[:],
                                blocked_dram_ap(qk_wT_d, P4, KP, 128, P4))
                            qk_t = [qk_all[:, p * P4 : (p + 1) * P4]
                                    for p in range(KP)]
                            nqcs_bc = bcast_dma(p1, nqcs_d, P4, "nqcs_bc")
                            if qk_bias:
                                qkb_bc = bcast_dma(p1, qkb_d, P4, "qkb_bc")
                            vb_t = part_bias_tiles(wper, v_b_d, "vb")
                            t2b_t = part_bias_tiles(wper, t2_b_d, "t2b")
                            m1b_t = part_bias_tiles(wper, m1_b_d, "m1b")
                            m2b_t = part_bias_tiles(wper, m2_b_d, "m2b")
                            rs1_t = part_bias_tiles(wper, m1_rs_d, "rs1")

                            t1b_holder = []

                            def deferred_weight_load(c):
                                if c == 0:
                                    nc.sync.dma_start(
                                        t1_all[:],
                                        blocked_dram_ap(t1_wT_d, P, KQ, 128, P))
                                elif c == 1:
                                    nc.sync.dma_start(
                                        t2_all[:],
                                        blocked_dram_ap(t2_wT_d, P, KQ, 128, P))
                                elif c == 2:
                                    t1b_holder.append(
                                        bcast_dma(wB12, t1_b_d, P, "t1b_bc"))
                                elif c == 3:
                                    nc.sync.dma_start(
                                        vw_all[:],
                                        blocked_dram_ap(vw_d, P, KP, 128, P))
                                elif c == 4:
                                    nc.sync.dma_start(
                                        m1_all[:],
                                        blocked_dram_ap(m1_wT_d, P, KP, 128, P))
                                elif c == 5:
                                    nc.sync.dma_start(
                                        m2_all[:],
                                        blocked_dram_ap(m2_wT_d, P, KP, 128, P))

                            def p1_stats_block(c):
                                xt_all = xt_pending.pop(c)
                                if c + 1 < NCH:
                                    xt_pending[c + 1] = xt_load(c + 1)
                                deferred_weight_load(c)
                                xt = [xt_all[:, p * CH : (p + 1) * CH]
                                      for p in range(KP)]
                                sq = []
                                for p in range(KP):
                                    s = p1.tile([128, CH], F32R, tag=f"sq{p}",
                                                name=f"sq{p}")
                                    nc.scalar.activation(s[:], xt[p][:], AF.Square)
                                    sq.append(s)
                                ps_s = ps1.tile([128, CH], F32, tag="ps_s",
                                                name="ps_s")
                                ps_q = ps1.tile([128, CH], F32, tag="ps_q",
                                                name="ps_q")
                                for p in range(KP):
                                    nc.tensor.matmul(ps_s[:], ones_s[:], xt[p][:],
                                                     start=(p == 0),
                                                     stop=(p == KP - 1))
                                for p in range(KP):
                                    nc.tensor.matmul(ps_q[:], ones_s[:], sq[p][:],
                                                     start=(p == 0),
                                                     stop=(p == KP - 1))
                                mu_b = p1.tile([128, CH], F32, tag="mu", name="mu",
                                               bufs=2)
                                nc.vector.tensor_copy(mu_b[:], ps_s[:])
                                var = p1.tile([128, CH], F32, tag="var",
                                              name="var", bufs=2)
                                nc.vector.tensor_mul(var[:], mu_b[:], mu_b[:])
                                nc.vector.tensor_tensor(var[:], ps_q[:], var[:],
                                                        ALU.subtract)
                                nc.scalar.activation(var[:], var[:], AF.Sqrt,
                                                     bias=eps_t[:])
                                rho_b = p1.tile([128, CH], F32, tag="rho",
                                                name="rho", bufs=2)
                                nc.vector.reciprocal(rho_b[:], var[:])
                                nc.gpsimd.tensor_copy(mu_st[c][:], mu_b[:])
                                nc.gpsimd.tensor_copy(rho_st[c][:], rho_b[:])
                                return xt, mu_b, rho_b

                            def p1_trans_block(c, mu_b, rho_b):
                                muT, rhoT = [], []
                                for ns in range(CH // 128):
                                    for src, dstl, nm in ((mu_b, muT, "muT"),
                                                          (rho_b, rhoT, "rhoT")):
                                        tr = ps1.tile([128, 128], F32, tag="tr",
                                                      name="tr_ps", bufs=2)
                                        nc.tensor.transpose(
                                            tr[:],
                                            src[:, ns * 128 : (ns + 1) * 128],
                                            ident_t[:])
                                        dt = p1.tile([128, 1], F32,
                                                     tag=f"{nm}{ns}",
                                                     name=f"{nm}{ns}", bufs=2)
                                        nc.vector.tensor_copy(dt[:],
                                                              tr[:, 0:1])
                                        dstl.append(dt)
                                return muT, rhoT

                            def p1_xq_block(c, xt, muT, rhoT):
                                for ns in range(CH // 128):
                                    xq_ps = ps1.tile([128, P4], F32, tag="xq",
                                                     name="xq_ps", bufs=2)
                                    for p in range(KP):
                                        nc.tensor.matmul(
                                            xq_ps[:],
                                            xt[p][:, ns * 128 : (ns + 1) * 128],
                                            qk_t[p][:],
                                            start=(p == 0), stop=(p == KP - 1))
                                    xq_sb = p1.tile([128, P4], F32R, tag="xqs",
                                                    name="xqs", bufs=4)
                                    nc.vector.scalar_tensor_tensor(
                                        xq_sb[:], nqcs_bc[:], muT[ns][:],
                                        xq_ps[:], op0=ALU.mult, op1=ALU.add)
                                    nc.gpsimd.tensor_scalar_mul(
                                        xq_sb[:], xq_sb[:], rhoT[ns][:])
                                    if qk_bias:
                                        nc.vector.tensor_tensor(
                                            xq_sb[:], xq_sb[:], qkb_bc[:],
                                            ALU.add)
                                    first = c == 0 and ns == 0
                                    last = c == NCH - 1 and ns == CH // 128 - 1
                                    for qh in range(KQ):
                                        nc.tensor.matmul(
                                            e_ps[qh],
                                            xq_sb[:, qh * 128 : (qh + 1) * 128],
                                            xq_sb[:],
                                            start=first, stop=last,
                                            skip_group_check=True)

                            prev = None
                            for c in range(NCH):
                                xt_c, mu_c, rho_c = p1_stats_block(c)
                                if prev is not None:
                                    p1_xq_block(c - 1, *prev)
                                trs = p1_trans_block(c, mu_c, rho_c)
                                prev = (xt_c, trs[0], trs[1])
                            p1_xq_block(NCH - 1, *prev)

                        # ========= PHASE B: logits, softmax, W', b' =========
                        with (
                            tc.tile_pool(name="pB", bufs=1) as pB,
                            tc.tile_pool(name="psB", bufs=1, space="PSUM") as psB,
                        ):
                            t1b_bc = t1b_holder[0]
                            energy_sb = []
                            for qh in range(KQ):
                                e = pB.tile([128, P4], F32R, tag=f"esb{qh}",
                                            name=f"esb{qh}")
                                nc.vector.tensor_copy(e[:], e_ps[qh])
                                energy_sb.append(e)

                            a1g = []
                            for bh in range(KQ):
                                a1 = pB.tile([128, P], F32, tag=f"a1_{bh}",
                                             name=f"a1_{bh}")
                                for oc in range(P // 512):
                                    ps = psB.tile([128, 512], F32, tag="a1ps",
                                                  name="a1_ps", bufs=2)
                                    for qh in range(KQ):
                                        nc.tensor.matmul(
                                            ps[:],
                                            energy_sb[qh][:, bh * 128 : (bh + 1) * 128],
                                            t1_t[qh][:, oc * 512 : (oc + 1) * 512],
                                            start=(qh == 0), stop=(qh == KQ - 1))
                                    nc.vector.tensor_tensor(
                                        a1[:, oc * 512 : (oc + 1) * 512], ps[:],
                                        t1b_bc[:, oc * 512 : (oc + 1) * 512],
                                        ALU.add)
                                ag = pB.tile([128, P], F32R, tag=f"a1g{bh}",
                                             name=f"a1g{bh}")
                                nc.scalar.activation(ag[:], a1[:], AF.Gelu)
                                a1g.append(ag)

                            att_t = [pB.tile([128, P], BF16, tag=f"att{o}",
                                             name=f"att{o}") for o in range(KP)]
                            bsum_ps = [psB.tile([128, 512], F32, tag="a1ps",
                                                name="a1_ps", bufs=2)
                                       for k in range(2)]
                            for o in range(KP):
                                att2f = pB.tile([128, P], F32, tag="attlg",
                                                name="attlg", bufs=2)
                                for kc in range(P // 512):
                                    ps = psB.tile([128, 512], F32, tag="a2ps",
                                                  name="a2_ps", bufs=2)
                                    for ph in range(KQ):
                                        nc.tensor.matmul(
                                            ps[:],
                                            t2_t[ph][:, o * 128 : (o + 1) * 128],
                                            a1g[ph][:, kc * 512 : (kc + 1) * 512],
                                            start=(ph == 0), stop=(ph == KQ - 1))
                                    nc.vector.tensor_scalar_add(
                                        att2f[:, kc * 512 : (kc + 1) * 512],
                                        ps[:], t2b_t[o][:])
                                negmax = pB.tile([128, 1], F32, tag="negmax",
                                                 name="negmax", bufs=2)
                                nc.vector.tensor_reduce(
                                    negmax[:], att2f[:], axis=AX.X, op=ALU.max,
                                    negate=True)
                                esum = pB.tile([128, 1], F32, tag="esum",
                                               name="esum", bufs=2)
                                expv = pB.tile([128, P], F32, tag="expv",
                                               name="expv")
                                nc.scalar.activation(
                                    expv[:], att2f[:], AF.Exp, bias=negmax[:],
                                    accum_out=esum[:])
                                rec = pB.tile([128, 1], F32, tag="rec",
                                              name="rec", bufs=2)
                                nc.vector.reciprocal(rec[:], esum[:])
                                nc.gpsimd.tensor_scalar_mul(att_t[o][:],
                                                              expv[:], rec[:])
                                recvb = pB.tile([128, 1], F32, tag="recvb",
                                                name="recvb", bufs=2)
                                nc.vector.tensor_mul(recvb[:], rec[:],
                                                     vb_t[o][:])
                                vbatt = pB.tile([128, P], F32R, tag="vbatt",
                                                name="vbatt")
                                nc.vector.tensor_scalar_mul(vbatt[:], expv[:],
                                                             recvb[:])
                                for k in range(2):
                                    nc.tensor.matmul(
                                        bsum_ps[k][:], ones1[:],
                                        vbatt[:, k * 512 : (k + 1) * 512],
                                        start=(o == 0), stop=(o == KP - 1),
                                        skip_group_check=True)

                            bp_bc = pB.tile([128, P], F32, tag="bc_rt",
                                            name="bc_rt")
                            for k in range(2):
                                nc.vector.tensor_copy(
                                    bp_bc[:, k * 512 : (k + 1) * 512],
                                    bsum_ps[k][:])
                            nc.gpsimd.dma_start(bp_dram[:], bp_bc[0:1, :])
                            bp_t = part_bias_tiles(wper, bp_dram, "bp", engine=nc.gpsimd)

                            for p in range(KP):
                                for qc in range(P // 512):
                                    ps = psB.tile([128, 512], F32, tag="wpps",
                                                  name="wp_ps", bufs=2)
                                    for o in range(KP):
                                        nc.tensor.matmul(
                                            ps[:],
                                            vw_t[o][:, p * 128 : (p + 1) * 128],
                                            att_t[o][:, qc * 512 : (qc + 1) * 512],
                                            start=(o == 0), stop=(o == KP - 1))
                                    nc.vector.tensor_copy(
                                        Wp[p][:, qc * 512 : (qc + 1) * 512],
                                        ps[:])

                            # csneg = -colsum(W') via -1 ones-matmul
                            cs_ps = [psB.tile([128, 512], F32, tag="a1ps",
                                              name="a1_ps", bufs=2)
                                     for k in range(2)]
                            for k in range(2):
                                for p in range(KP):
                                    nc.tensor.matmul(
                                        cs_ps[k][:], onesm1[:],
                                        Wp[p][:, k * 512 : (k + 1) * 512],
                                        start=(p == 0), stop=(p == KP - 1),
                                        skip_group_check=True)
                            cs_bc = pB.tile([128, P], F32, tag="bc_rt",
                                            name="bc_rt")
                            for k in range(2):
                                nc.vector.tensor_copy(
                                    cs_bc[:, k * 512 : (k + 1) * 512],
                                    cs_ps[k][:])
                            nc.gpsimd.dma_start(cs_dram[:], cs_bc[0:1, :])
                            csn_t = part_bias_tiles(wper, cs_dram, "csn", engine=nc.gpsimd)

                # ========= PASS 2: t_out, x1, folded-LN2 MLP, out =========
                with (
                    tc.tile_pool(name="p2", bufs=1) as p2,
                    tc.tile_pool(name="ps2", bufs=1, space="PSUM") as ps2,
                ):
                    def xt2_load(c):
                        t = px.tile([128, KP * CH], F32R, tag="xt",
                                    name="xt", bufs=2)
                        nc.sync.dma_start(
                            t[:],
                            blocked_dram_ap(xT_d, N, KP, 128, CH,
                                            col_off=c * CH))
                        return t

                    xt2_pending = {0: xt2_load(0)}
                    for c in range(NCH):
                        cs = slice(c * CH, (c + 1) * CH)
                        xt2_all = xt2_pending.pop(c)
                        if c + 1 < NCH:
                            xt2_pending[c + 1] = xt2_load(c + 1)
                        xt2 = [xt2_all[:, p * CH : (p + 1) * CH]
                               for p in range(KP)]
                        muf = p2.tile([128, CH], F32, tag="muf", name="muf",
                                      bufs=2)
                        nc.gpsimd.tensor_copy(muf[:], mu_st[c][:])
                        rhof = p2.tile([128, CH], F32, tag="rhof", name="rhof",
                                       bufs=2)
                        nc.gpsimd.tensor_copy(rhof[:], rho_st[c][:])

                        ps_s2 = ps2.tile([128, CH], F32, tag="ps_s2",
                                         name="ps_s2")
                        ps_q2 = ps2.tile([128, CH], F32, tag="ps_q2",
                                         name="ps_q2")
                        x1l, x1bl, sq2l = [], [], []

                        def stats_mms(i):
                            nc.tensor.matmul(ps_s2[:], ones_sb[:], x1bl[i][:],
                                             start=(i == 0), stop=(i == KP - 1),
                                             skip_group_check=True)
                            nc.tensor.matmul(ps_q2[:], ones_sb[:], sq2l[i][:],
                                             start=(i == 0), stop=(i == KP - 1),
                                             skip_group_check=True)

                        for q in range(KP):
                            tout_ps = ps2.tile([128, CH], F32, tag="tout",
                                               name="tout_ps", bufs=2)
                            for p in range(KP):
                                nc.tensor.matmul(
                                    tout_ps[:],
                                    Wp[p][:, q * 128 : (q + 1) * 128],
                                    xt2[p][:],
                                    start=(p == 0), stop=(p == KP - 1))
                            xs = p2.tile([128, CH], F32, tag="xs", name="xs",
                                         bufs=3)
                            nc.vector.scalar_tensor_tensor(
                                xs[:], muf[:], csn_t[q][:], tout_ps[:],
                                op0=ALU.mult, op1=ALU.add)
                            nc.gpsimd.tensor_mul(xs[:], xs[:], rhof[:])
                            x1q = p2.tile([128, CH], F32, tag=f"x1_{q}",
                                          name=f"x1_{q}")
                            nc.vector.scalar_tensor_tensor(
                                x1q[:], xs[:], bp_t[q][:], xt2[q][:],
                                op0=ALU.add, op1=ALU.add)
                            x1b = p2.tile([128, CH], BF16, tag=f"x1b{q}",
                                          name=f"x1b{q}")
                            nc.gpsimd.tensor_copy(x1b[:], x1q[:])
                            sq2 = p2.tile([128, CH], BF16, tag=f"sq2{q}",
                                          name=f"sq2{q}")
                            nc.scalar.activation(sq2[:], x1q[:], AF.Square)
                            x1l.append(x1q); x1bl.append(x1b); sq2l.append(sq2)
                            if q >= 1:
                                stats_mms(q - 1)
                        stats_mms(KP - 1)

                        negmu2 = p2.tile([128, CH], F32, tag="negmu2",
                                         name="negmu2", bufs=2)
                        nc.vector.tensor_scalar_mul(negmu2[:], ps_s2[:], -1.0)
                        var2 = p2.tile([128, CH], F32, tag="var2", name="var2",
                                       bufs=2)
                        nc.vector.tensor_mul(var2[:], negmu2[:], negmu2[:])
                        nc.vector.tensor_tensor(var2[:], ps_q2[:], var2[:],
                                                ALU.subtract)
                        nc.scalar.activation(var2[:], var2[:], AF.Sqrt,
                                             bias=eps_t[:])
                        rho2 = p2.tile([128, CH], F32, tag="rho2", name="rho2",
                                       bufs=2)
                        nc.vector.reciprocal(rho2[:], var2[:])

                        mg = []
                        for j in range(KP):
                            m1_ps = ps2.tile([128, CH], F32, tag="m1ps",
                                             name="m1_ps", bufs=2)
                            for p in range(KP):
                                nc.tensor.matmul(
                                    m1_ps[:],
                                    m1_t[p][:, j * 128 : (j + 1) * 128],
                                    x1bl[p][:],
                                    start=(p == 0), stop=(p == KP - 1))
                            ct = p2.tile([128, CH], F32, tag="ct", name="ct",
                                         bufs=3)
                            nc.vector.scalar_tensor_tensor(
                                ct[:], negmu2[:], rs1_t[j][:], m1_ps[:],
                                op0=ALU.mult, op1=ALU.add)
                            nc.vector.tensor_mul(ct[:], ct[:], rho2[:])
                            g = p2.tile([128, CH], BF16, tag=f"mg{j}",
                                        name=f"mg{j}")
                            nc.scalar.activation(g[:], ct[:], AF.Gelu,
                                                 bias=m1b_t[j][:])
                            mg.append(g)

                        mo_all = p2.tile([128, KP * CH], F32, tag="mo",
                                         name="mo")
                        for o in range(KP):
                            m2_ps = ps2.tile([128, CH], F32, tag="m2ps",
                                             name="m2_ps", bufs=2)
                            for j in range(KP):
                                nc.tensor.matmul(
                                    m2_ps[:],
                                    m2_t[j][:, o * 128 : (o + 1) * 128],
                                    mg[j][:],
                                    start=(j == 0), stop=(j == KP - 1))
                            nc.vector.scalar_tensor_tensor(
                                mo_all[:, o * CH : (o + 1) * CH], m2_ps[:],
                                m2b_t[o][:], x1l[o][:],
                                op0=ALU.add, op1=ALU.add)
                        nc.sync.dma_start(
                            blocked_dram_ap(outT_d, N, KP, 128, CH,
                                            col_off=c * CH),
                            mo_all[:])

            _loop_ctx.close()

    nc.compile()
    return nc


_CACHE = {}


def _get_nc(qk_bias, loop_R=1):
    key = (qk_bias, loop_R)
    if key not in _CACHE:
        _CACHE[key] = _build(qk_bias, loop_R)
    return _CACHE[key]


def _round_f32r(x):
    u = np.ascontiguousarray(x, np.float32).view(np.uint32)
    shift = 13
    bias = np.uint32((1 << (shift - 1)) - 1)
    lsb = (u >> np.uint32(shift)) & np.uint32(1)
    u2 = (u + bias + lsb) & np.uint32(~((1 << shift) - 1) & 0xFFFFFFFF)
    return u2.view(np.float32)


def kernel(**inputs):
    return _kernel_impl(inputs, loop_R=1)


def _kernel_impl(inputs, loop_R=1):
    x = np.ascontiguousarray(np.asarray(inputs["x"], np.float32))
    assert x.shape == (B, N, P), x.shape

    f32 = lambda k: np.asarray(inputs[k], np.float32)
    ln1_g, ln1_b = f32("ln1_g"), f32("ln1_b")
    ln2_g, ln2_b = f32("ln2_g"), f32("ln2_b")
    qk_w, v_w, v_b = f32("qk_w"), f32("v_w"), f32("v_b")
    t1_w, t1_b = f32("t1_w"), f32("t1_b")
    t2_w, t2_b = f32("t2_w"), f32("t2_b")
    m1_w, m1_b = f32("m1_w"), f32("m1_b")
    m2_w, m2_b = f32("m2_w"), f32("m2_b")

    qk_w_eff = qk_w * ln1_g[None, :]
    v_w_eff = v_w * ln1_g[None, :]
    v_b_eff = v_b + v_w @ ln1_b
    qk_b = qk_w @ ln1_b
    qk_bias = bool(np.any(qk_b != 0.0))
    m1_w_eff = m1_w * ln2_g[None, :]
    m1_b_eff = m1_b + m1_w @ ln2_b

    nc = _get_nc(qk_bias, loop_R)

    bf = ml_dtypes.bfloat16
    m1_bf = np.ascontiguousarray(m1_w_eff.T).astype(bf)
    m1_rs = m1_bf.astype(np.float32).sum(axis=0)
    base = {
        "qk_wT": _round_f32r(qk_w_eff.T),
        "vw": np.ascontiguousarray(v_w_eff).astype(bf),
        "t1_wT": _round_f32r(t1_w.T),
        "t2_wT": _round_f32r(t2_w.T),
        "m1_wT": m1_bf,
        "m2_wT": np.ascontiguousarray(m2_w.T).astype(bf),
        "v_b": np.ascontiguousarray(v_b_eff),
        "t1_b": np.ascontiguousarray(t1_b),
        "t2_b": np.ascontiguousarray(t2_b),
        "m1_b": np.ascontiguousarray(m1_b_eff),
        "m2_b": np.ascontiguousarray(m2_b),
        "m1_rs": np.ascontiguousarray(m1_rs),
        "nqcs": np.ascontiguousarray(-_round_f32r(qk_w_eff.T).sum(axis=0)),
        "ident": np.eye(128, dtype=np.float32),
    }
    if qk_bias:
        base["qk_b"] = np.ascontiguousarray(qk_b)

    in_maps = []
    for b in range(B):
        m = dict(base)
        m["xT"] = np.ascontiguousarray(x[b].T)
        in_maps.append(m)

    res = bass_utils.run_bass_kernel_spmd(nc, in_maps, core_ids=list(range(B)))
    out = np.empty((B, N, P), np.float32)
    for b in range(B):
        out[b] = res.results[b]["outT"].T
    return out


# revision 3
# speedup vs baseline: 4.6907x; 4.6907x over previous
"""Trainium2 Bass kernel for nn_Enhanced_transformer (dense transformer block).

v9 = v8 + ring discipline: every load (weights incl.) rides the SP ring,
interleaved one weight blob per pass-1 chunk behind the x prefetch; x
chunk loads are issued one chunk ahead in both passes; out-stores go on
SP after the next chunk's prefetch. att2f bias-add and negmu move to DVE
(fewer ACT activation-table swaps).
v8 = v7 + phase-boundary overlap fixes: att/v_w in bf16, v_w + t1_b
preloaded during pass 1, x-chunk tiles in a cross-phase pool (pass-2
prefetch starts during phase B), b'/cs round-trips on the gpsimd ring so
they never block ACT compute.
v7 = v6 + LN1 also folded into the x_q GEMM: x_q = rho_t*((x@qk) -
mu_t*colsum(qk)); the token-partition-oriented mu/rho come from PE
transposes of the broadcast stats (identity matrix shipped as an input).
No h tensor anywhere; every GEMM runs on raw f32r x.
v6 = v5 + batched DMA (one multi-level-AP descriptor per chunk / weight /
bias vector instead of per-tile triggers; a dma_start costs ~0.7us of issuing
engine sequencer time and big descriptors run at full fabric bandwidth).
v5 = v4 + LN1 folded into the t_out GEMM (no h in pass 2 at all):
  t_out = rho * (x @ W' - mu * colsum(W')) ; colsum via -1-matmul + tiny
  DRAM round-trip (same pattern as b'). Pass 2 GEMMs directly on the raw
  f32r x tiles that the residual needs anyway, so per-chunk DMA drops to
  x-in + out-out and pass 1 only streams x once (no h spill).
- mu/rho kept in SBUF (bf16) from pass 1; LN2 folded into m1 as in v3
- DMA rings split: SP = bulk loads only, ACT = weights + b'/cs + stores
See kernel_v2 docstring for the base structure (W' fusion, f32r logits, no
DRAM spills). Numpy-faithful dtype sim: L2 ~5.8e-3 (gate 2e-2).
"""

import numpy as np
import ml_dtypes

import concourse.bass as bass
import concourse.tile as tile
from concourse import bacc, mybir
from concourse import bass_utils

F32 = mybir.dt.float32
F32R = mybir.dt.float32r
BF16 = mybir.dt.bfloat16
AF = mybir.ActivationFunctionType
ALU = mybir.AluOpType
AX = mybir.AxisListType

B, N, P = 8, 4096, 1024
P4 = P // 4          # 256
EPS = 1e-5
CH = 512             # token chunk
NCH = N // CH        # 8
KP = P // 128        # 8 channel tiles
KQ = P4 // 128       # 2


def _build(qk_bias: bool, loop_R: int = 1):
    nc = bacc.Bacc("TRN2", target_bir_lowering=False, debug=False)

    xT_d = nc.dram_tensor("xT", [P, N], F32R, kind="ExternalInput").ap()
    qk_wT_d = nc.dram_tensor("qk_wT", [P, P4], F32R, kind="ExternalInput").ap()
    vw_d = nc.dram_tensor("vw", [P, P], BF16, kind="ExternalInput").ap()
    t1_wT_d = nc.dram_tensor("t1_wT", [P4, P], F32R, kind="ExternalInput").ap()
    t2_wT_d = nc.dram_tensor("t2_wT", [P4, P], F32R, kind="ExternalInput").ap()
    m1_wT_d = nc.dram_tensor("m1_wT", [P, P], BF16, kind="ExternalInput").ap()
    m2_wT_d = nc.dram_tensor("m2_wT", [P, P], BF16, kind="ExternalInput").ap()
    v_b_d = nc.dram_tensor("v_b", [P], F32, kind="ExternalInput").ap()
    t1_b_d = nc.dram_tensor("t1_b", [P], F32, kind="ExternalInput").ap()
    t2_b_d = nc.dram_tensor("t2_b", [P], F32, kind="ExternalInput").ap()
    m1_b_d = nc.dram_tensor("m1_b", [P], F32, kind="ExternalInput").ap()
    m2_b_d = nc.dram_tensor("m2_b", [P], F32, kind="ExternalInput").ap()
    m1_rs_d = nc.dram_tensor("m1_rs", [P], F32, kind="ExternalInput").ap()
    nqcs_d = nc.dram_tensor("nqcs", [P4], F32, kind="ExternalInput").ap()
    ident_d = nc.dram_tensor("ident", [128, 128], F32, kind="ExternalInput").ap()
    if qk_bias:
        qkb_d = nc.dram_tensor("qk_b", [P4], F32, kind="ExternalInput").ap()
    outT_d = nc.dram_tensor("outT", [P, N], F32, kind="ExternalOutput").ap()

    def part_bias_tiles(pool, dram_ap, name, engine=None):
        """[P] dram vector -> one [128, KP] tile, one DMA; returns slices."""
        bt = pool.tile([128, KP], F32, tag=name, name=name)
        src = bass.AP(tensor=dram_ap.tensor, offset=dram_ap.offset,
                      ap=[[1, 128], [128, KP]])
        (engine or nc.sync).dma_start(bt[:], src)
        return [bt[:, t : t + 1] for t in range(KP)]

    def blocked_dram_ap(dram_ap, row_stride, nblk, blk_rows, width, col_off=0):
        """AP iterating (q in 128, blk in nblk, j in width):
        dram[blk*blk_rows + q, col_off + j] for a row-major [rows, row_stride]
        dram tensor."""
        return bass.AP(
            tensor=dram_ap.tensor,
            offset=dram_ap.offset + col_off,
            ap=[[row_stride, 128], [row_stride * blk_rows, nblk], [1, width]],
        )

    def bcast_dma(pool, dram_ap, width, name):
        bt = pool.tile([128, width], F32, tag=name, name=name)
        src = bass.AP(
            tensor=dram_ap.tensor, offset=dram_ap.offset,
            ap=[[0, 128], *dram_ap.ap],
        )
        nc.sync.dma_start(bt[:], src)
        return bt

    with tile.TileContext(nc) as tc:
        with (
            tc.tile_pool(name="dram", bufs=1, space="DRAM") as dramp,
            tc.tile_pool(name="consts", bufs=1) as consts,
        ):
            bp_dram = dramp.tile([P], F32, name="bp_dram")
            cs_dram = dramp.tile([P], F32, name="cs_dram")

            ones_f = consts.tile([128, 128], F32, tag="ones_f", name="ones_f")
            nc.vector.memset(ones_f[:], 1.0 / P)
            ones_s = consts.tile([128, 128], F32R, tag="ones_s", name="ones_s")
            nc.vector.tensor_copy(ones_s[:], ones_f[:])
            ones_sb = consts.tile([128, 128], BF16, tag="ones_sb", name="ones_sb")
            nc.vector.tensor_copy(ones_sb[:], ones_f[:])
            ones_g = consts.tile([128, 128], F32, tag="ones_g", name="ones_g")
            nc.vector.memset(ones_g[:], 1.0)
            ones1 = consts.tile([128, 128], F32R, tag="ones1", name="ones1")
            nc.vector.tensor_copy(ones1[:], ones_g[:])
            ones_h = consts.tile([128, 128], F32, tag="ones_h", name="ones_h")
            nc.vector.memset(ones_h[:], -1.0)
            onesm1 = consts.tile([128, 128], F32R, tag="onesm1", name="onesm1")
            nc.vector.tensor_copy(onesm1[:], ones_h[:])
            eps_t = consts.tile([128, 1], F32, tag="eps", name="eps_t")
            nc.vector.memset(eps_t[:], EPS)
            ident_t = consts.tile([128, 128], F32, tag="ident", name="ident")
            nc.sync.dma_start(ident_t[:], ident_d[:, :])
            mu_st = [consts.tile([128, CH], BF16, tag=f"must{c}",
                                 name=f"must{c}") for c in range(NCH)]
            rho_st = [consts.tile([128, CH], BF16, tag=f"rhst{c}",
                                  name=f"rhst{c}") for c in range(NCH)]

            from contextlib import ExitStack as _ES
            _loop_ctx = _ES()
            if loop_R > 1:
                _loop_ctx.enter_context(tc.For_i(0, loop_R, 1))

            with (
                tc.tile_pool(name="wper", bufs=1) as wper,
                tc.tile_pool(name="px", bufs=1) as px,
            ):
                # allocate persistent weight tiles; DMAs are issued later in
                # priority order (qk -> t1/t2 -> m1/m2)
                m1_all = wper.tile([128, KP * P], BF16, tag="m1w", name="m1w")
                m1_t = [m1_all[:, p * P : (p + 1) * P] for p in range(KP)]
                m2_all = wper.tile([128, KP * P], BF16, tag="m2w", name="m2w")
                m2_t = [m2_all[:, p * P : (p + 1) * P] for p in range(KP)]
                Wp = [wper.tile([128, P], F32R, tag=f"Wp{p}", name=f"Wp{p}")
                      for p in range(KP)]

                with tc.tile_pool(name="wB12", bufs=1) as wB12:
                    t1_all = wB12.tile([128, KQ * P], F32R, tag="t1w", name="t1w")
                    t1_t = [t1_all[:, qh * P : (qh + 1) * P] for qh in range(KQ)]
                    t2_all = wB12.tile([128, KQ * P], F32R, tag="t2w", name="t2w")
                    t2_t = [t2_all[:, qh * P : (qh + 1) * P] for qh in range(KQ)]
                    vw_all = wB12.tile([128, KP * P], BF16, tag="vw", name="vw")
                    vw_t = [vw_all[:, o * P : (o + 1) * P] for o in range(KP)]

                    with tc.tile_pool(name="psE", bufs=1, space="PSUM") as psE:
                        e_ps = [psE.tile([128, P4], F32, tag=f"e{i}",
                                         name=f"e_ps{i}")[:] for i in range(KQ)]

                        # ========= PASS 1: LN1 stats, x_q, energy =========
                        with (
                            tc.tile_pool(name="p1", bufs=1) as p1,
                            tc.tile_pool(name="ps1", bufs=1, space="PSUM") as ps1,
                        ):
                            def xt_load(c, split=False):
                                t = px.tile([128, KP * CH], F32R,
                                            tag="xt", name="xt", bufs=2)
                                if split:
                                    half = KP // 2
                                    for hb in range(2):
                                        src = bass.AP(
                                            tensor=xT_d.tensor,
                                            offset=(xT_d.offset + c * CH
                                                    + hb * half * 128 * N),
                                            ap=[[N, 128], [128 * N, half],
                                                [1, CH]],
                                        )
                                        nc.sync.dma_start(
                                            t[:, hb * half * CH
                                              : (hb + 1) * half * CH], src)
                                else:
                                    nc.sync.dma_start(
                                        t[:],
                                        blocked_dram_ap(xT_d, N, KP, 128, CH,
                                                        col_off=c * CH))
                                return t

                            xt_pending = {0: xt_load(0, split=True)}
                            qk_all = p1.tile([128, KP * P4], F32R, tag="qkw",
                                             name="qkw")
                            nc.sync.dma_start(
                                qk_all[:],
                                blocked_dram_ap(qk_wT_d, P4, KP, 128, P4))
                            qk_t = [qk_all[:, p * P4 : (p + 1) * P4]
                                    for p in range(KP)]
                            nqcs_bc = bcast_dma(p1, nqcs_d, P4, "nqcs_bc")
                            if qk_bias:
                                qkb_bc = bcast_dma(p1, qkb_d, P4, "qkb_bc")
                            vb_t = part_bias_tiles(wper, v_b_d, "vb")
                            t2b_t = part_bias_tiles(wper, t2_b_d, "t2b")
                            m1b_t = part_bias_tiles(wper, m1_b_d, "m1b")
                            m2b_t = part_bias_tiles(wper, m2_b_d, "m2b")
                            rs1_t = part_bias_tiles(wper, m1_rs_d, "rs1")

                            t1b_holder = []

                            def deferred_weight_load(c):
                                if c == 0:
                                    nc.sync.dma_start(
                                        t1_all[:],
                                        blocked_dram_ap(t1_wT_d, P, KQ, 128, P))
                                elif c == 1:
                                    nc.sync.dma_start(
                                        t2_all[:],
                                        blocked_dram_ap(t2_wT_d, P, KQ, 128, P))
                                elif c == 2:
                                    t1b_holder.append(
                                        bcast_dma(wB12, t1_b_d, P, "t1b_bc"))
                                elif c == 3:
                                    nc.sync.dma_start(
                                        vw_all[:],
                                        blocked_dram_ap(vw_d, P, KP, 128, P))
                                elif c == 4:
                                    nc.sync.dma_start(
                                        m1_all[:],
                                        blocked_dram_ap(m1_wT_d, P, KP, 128, P))
                                elif c == 5:
                                    nc.sync.dma_start(
                                        m2_all[:],
                                        blocked_dram_ap(m2_wT_d, P, KP, 128, P))

                            def p1_stats_block(c):
                                xt_all = xt_pending.pop(c)
                                if c + 1 < NCH:
                                    xt_pending[c + 1] = xt_load(c + 1)
                                deferred_weight_load(c)
                                xt = [xt_all[:, p * CH : (p + 1) * CH]
                                      for p in range(KP)]
                                sq = []
                                for p in range(KP):
                                    s = p1.tile([128, CH], F32R, tag=f"sq{p}",
                                                name=f"sq{p}")
                                    nc.scalar.activation(s[:], xt[p][:], AF.Square)
                                    sq.append(s)
                                ps_s = ps1.tile([128, CH], F32, tag="ps_s",
                                                name="ps_s")
                                ps_q = ps1.tile([128, CH], F32, tag="ps_q",
                                                name="ps_q", bufs=2)
                                for p in range(KP):
                                    nc.tensor.matmul(ps_s[:], ones_s[:], xt[p][:],
                                                     start=(p == 0),
                                                     stop=(p == KP - 1))
                                for p in range(KP):
                                    nc.tensor.matmul(ps_q[:], ones_s[:], sq[p][:],
                                                     start=(p == 0),
                                                     stop=(p == KP - 1))
                                mu_b = p1.tile([128, CH], F32, tag="mu", name="mu",
                                               bufs=2)
                                nc.vector.tensor_copy(mu_b[:], ps_s[:])
                                var = p1.tile([128, CH], F32, tag="var",
                                              name="var", bufs=2)
                                nc.vector.tensor_mul(var[:], mu_b[:], mu_b[:])
                                nc.vector.tensor_tensor(var[:], ps_q[:], var[:],
                                                        ALU.subtract)
                                nc.scalar.activation(var[:], var[:], AF.Sqrt,
                                                     bias=eps_t[:])
                                rho_b = p1.tile([128, CH], F32, tag="rho",
                                                name="rho", bufs=2)
                                nc.vector.reciprocal(rho_b[:], var[:])
                                nc.gpsimd.tensor_copy(mu_st[c][:], mu_b[:])
                                nc.gpsimd.tensor_copy(rho_st[c][:], rho_b[:])
                                return xt, mu_b, rho_b

                            def p1_trans_block(c, mu_b, rho_b):
                                muT, rhoT = [], []
                                for ns in range(CH // 128):
                                    for src, dstl, nm in ((mu_b, muT, "muT"),
                                                          (rho_b, rhoT, "rhoT")):
                                        tr = ps1.tile([128, 128], F32, tag="tr",
                                                      name="tr_ps")
                                        nc.tensor.transpose(
                                            tr[:],
                                            src[:, ns * 128 : (ns + 1) * 128],
                                            ident_t[:])
                                        dt = p1.tile([128, 1], F32,
                                                     tag=f"{nm}{ns}",
                                                     name=f"{nm}{ns}", bufs=2)
                                        nc.vector.tensor_copy(dt[:],
                                                              tr[:, 0:1])
                                        dstl.append(dt)
                                return muT, rhoT

                            def p1_xq_block(c, xt, muT, rhoT):
                                for ns in range(CH // 128):
                                    xq_ps = ps1.tile([128, P4], F32, tag="xq",
                                                     name="xq_ps", bufs=2)
                                    for p in range(KP):
                                        nc.tensor.matmul(
                                            xq_ps[:],
                                            xt[p][:, ns * 128 : (ns + 1) * 128],
                                            qk_t[p][:],
                                            start=(p == 0), stop=(p == KP - 1))
                                    xq_sb = p1.tile([128, P4], F32R, tag="xqs",
                                                    name="xqs", bufs=4)
                                    nc.vector.scalar_tensor_tensor(
                                        xq_sb[:], nqcs_bc[:], muT[ns][:],
                                        xq_ps[:], op0=ALU.mult, op1=ALU.add)
                                    nc.gpsimd.tensor_scalar_mul(
                                        xq_sb[:], xq_sb[:], rhoT[ns][:])
                                    if qk_bias:
                                        nc.vector.tensor_tensor(
                                            xq_sb[:], xq_sb[:], qkb_bc[:],
                                            ALU.add)
                                    first = c == 0 and ns == 0
                                    last = c == NCH - 1 and ns == CH // 128 - 1
                                    for qh in range(KQ):
                                        nc.tensor.matmul(
                                            e_ps[qh],
                                            xq_sb[:, qh * 128 : (qh + 1) * 128],
                                            xq_sb[:],
                                            start=first, stop=last,
                                            skip_group_check=True)

                            prev = None
                            for c in range(NCH):
                                xt_c, mu_c, rho_c = p1_stats_block(c)
                                if prev is not None:
                                    p1_xq_block(c - 1, *prev)
                                trs = p1_trans_block(c, mu_c, rho_c)
                                prev = (xt_c, trs[0], trs[1])
                            p1_xq_block(NCH - 1, *prev)

                        # ========= PHASE B: logits, softmax, W', b' =========
                        with (
                            tc.tile_pool(name="pB", bufs=1) as pB,
                            tc.tile_pool(name="psB", bufs=1, space="PSUM") as psB,
                        ):
                            t1b_bc = t1b_holder[0]
                            energy_sb = []
                            for qh in range(KQ):
                                e = pB.tile([128, P4], F32R, tag=f"esb{qh}",
                                            name=f"esb{qh}")
                                nc.vector.tensor_copy(e[:], e_ps[qh])
                                energy_sb.append(e)

                            a1g = []
                            for bh in range(KQ):
                                a1 = pB.tile([128, P], F32, tag=f"a1_{bh}",
                                             name=f"a1_{bh}")
                                for oc in range(P // 512):
                                    ps = psB.tile([128, 512], F32, tag="a1ps",
                                                  name="a1_ps", bufs=2)
                                    for qh in range(KQ):
                                        nc.tensor.matmul(
                                            ps[:],
                                            energy_sb[qh][:, bh * 128 : (bh + 1) * 128],
                                            t1_t[qh][:, oc * 512 : (oc + 1) * 512],
                                            start=(qh == 0), stop=(qh == KQ - 1))
                                    nc.vector.tensor_tensor(
                                        a1[:, oc * 512 : (oc + 1) * 512], ps[:],
                                        t1b_bc[:, oc * 512 : (oc + 1) * 512],
                                        ALU.add)
                                ag = pB.tile([128, P], F32R, tag=f"a1g{bh}",
                                             name=f"a1g{bh}")
                                nc.scalar.activation(ag[:], a1[:], AF.Gelu)
                                a1g.append(ag)

                            att_t = [pB.tile([128, P], BF16, tag=f"att{o}",
                                             name=f"att{o}") for o in range(KP)]
                            bsum_ps = [psB.tile([128, 512], F32, tag="a1ps",
                                                name="a1_ps", bufs=2)
                                       for k in range(2)]
                            for o in range(KP):
                                att2f = pB.tile([128, P], F32, tag="attlg",
                                                name="attlg", bufs=2)
                                for kc in range(P // 512):
                                    ps = psB.tile([128, 512], F32, tag="a2ps",
                                                  name="a2_ps", bufs=2)
                                    for ph in range(KQ):
                                        nc.tensor.matmul(
                                            ps[:],
                                            t2_t[ph][:, o * 128 : (o + 1) * 128],
                                            a1g[ph][:, kc * 512 : (kc + 1) * 512],
                                            start=(ph == 0), stop=(ph == KQ - 1))
                                    nc.vector.tensor_scalar_add(
                                        att2f[:, kc * 512 : (kc + 1) * 512],
                                        ps[:], t2b_t[o][:])
                                negmax = pB.tile([128, 1], F32, tag="negmax",
                                                 name="negmax", bufs=2)
                                nc.vector.tensor_reduce(
                                    negmax[:], att2f[:], axis=AX.X, op=ALU.max,
                                    negate=True)
                                esum = pB.tile([128, 1], F32, tag="esum",
                                               name="esum", bufs=2)
                                expv = pB.tile([128, P], F32, tag="expv",
                                               name="expv")
                                nc.scalar.activation(
                                    expv[:], att2f[:], AF.Exp, bias=negmax[:],
                                    accum_out=esum[:])
                                rec = pB.tile([128, 1], F32, tag="rec",
                                              name="rec", bufs=2)
                                nc.vector.reciprocal(rec[:], esum[:])
                                nc.gpsimd.tensor_scalar_mul(att_t[o][:],
                                                              expv[:], rec[:])
                                recvb = pB.tile([128, 1], F32, tag="recvb",
                                                name="recvb", bufs=2)
                                nc.vector.tensor_mul(recvb[:], rec[:],
                                                     vb_t[o][:])
                                vbatt = pB.tile([128, P], F32R, tag="vbatt",
                                                name="vbatt")
                                nc.vector.tensor_scalar_mul(vbatt[:], expv[:],
                                                             recvb[:])
                                for k in range(2):
                                    nc.tensor.matmul(
                                        bsum_ps[k][:], ones1[:],
                                        vbatt[:, k * 512 : (k + 1) * 512],
                                        start=(o == 0), stop=(o == KP - 1),
                                        skip_group_check=True)

                            bp_bc = pB.tile([128, P], F32, tag="bc_rt",
                                            name="bc_rt")
                            for k in range(2):
                                nc.vector.tensor_copy(
                                    bp_bc[:, k * 512 : (k + 1) * 512],
                                    bsum_ps[k][:])
                            nc.gpsimd.dma_start(bp_dram[:], bp_bc[0:1, :])
                            bp_t = part_bias_tiles(wper, bp_dram, "bp", engine=nc.gpsimd)

                            for p in range(KP):
                                for qc in range(P // 512):
                                    ps = psB.tile([128, 512], F32, tag="wpps",
                                                  name="wp_ps", bufs=2)
                                    for o in range(KP):
                                        nc.tensor.matmul(
                                            ps[:],
                                            vw_t[o][:, p * 128 : (p + 1) * 128],
                                            att_t[o][:, qc * 512 : (qc + 1) * 512],
                                            start=(o == 0), stop=(o == KP - 1))
                                    nc.vector.tensor_copy(
                                        Wp[p][:, qc * 512 : (qc + 1) * 512],
                                        ps[:])

                            # csneg = -colsum(W') via -1 ones-matmul
                            cs_ps = [psB.tile([128, 512], F32, tag="a1ps",
                                              name="a1_ps", bufs=2)
                                     for k in range(2)]
                            for k in range(2):
                                for p in range(KP):
                                    nc.tensor.matmul(
                                        cs_ps[k][:], onesm1[:],
                                        Wp[p][:, k * 512 : (k + 1) * 512],
                                        start=(p == 0), stop=(p == KP - 1),
                                        skip_group_check=True)
                            cs_bc = pB.tile([128, P], F32, tag="bc_rt",
                                            name="bc_rt")
                            for k in range(2):
                                nc.vector.tensor_copy(
                                    cs_bc[:, k * 512 : (k + 1) * 512],
                                    cs_ps[k][:])
                            nc.gpsimd.dma_start(cs_dram[:], cs_bc[0:1, :])
                            csn_t = part_bias_tiles(wper, cs_dram, "csn", engine=nc.gpsimd)

                # ========= PASS 2: t_out, x1, folded-LN2 MLP, out =========
                with (
                    tc.tile_pool(name="p2", bufs=1) as p2,
                    tc.tile_pool(name="ps2", bufs=1, space="PSUM") as ps2,
                ):
                    def xt2_load(c):
                        t = px.tile([128, KP * CH], F32R, tag="xt",
                                    name="xt", bufs=2)
                        nc.sync.dma_start(
                            t[:],
                            blocked_dram_ap(xT_d, N, KP, 128, CH,
                                            col_off=c * CH))
                        return t

                    xt2_pending = {0: xt2_load(0)}
                    for c in range(NCH):
                        cs = slice(c * CH, (c + 1) * CH)
                        xt2_all = xt2_pending.pop(c)
                        if c + 1 < NCH:
                            xt2_pending[c + 1] = xt2_load(c + 1)
                        xt2 = [xt2_all[:, p * CH : (p + 1) * CH]
                               for p in range(KP)]
                        muf = p2.tile([128, CH], F32, tag="muf", name="muf",
                                      bufs=2)
                        nc.gpsimd.tensor_copy(muf[:], mu_st[c][:])
                        rhof = p2.tile([128, CH], F32, tag="rhof", name="rhof",
                                       bufs=2)
                        nc.gpsimd.tensor_copy(rhof[:], rho_st[c][:])

                        ps_s2 = ps2.tile([128, CH], F32, tag="ps_s2",
                                         name="ps_s2")
                        ps_q2 = ps2.tile([128, CH], F32, tag="ps_q2",
                                         name="ps_q2")
                        x1l, x1bl, sq2l = [], [], []

                        def stats_mms(i):
                            nc.tensor.matmul(ps_s2[:], ones_sb[:], x1bl[i][:],
                                             start=(i == 0), stop=(i == KP - 1),
                                             skip_group_check=True)
                            nc.tensor.matmul(ps_q2[:], ones_sb[:], sq2l[i][:],
                                             start=(i == 0), stop=(i == KP - 1),
                                             skip_group_check=True)

                        for q in range(KP):
                            tout_ps = ps2.tile([128, CH], F32, tag="tout",
                                               name="tout_ps", bufs=2)
                            for p in range(KP):
                                nc.tensor.matmul(
                                    tout_ps[:],
                                    Wp[p][:, q * 128 : (q + 1) * 128],
                                    xt2[p][:],
                                    start=(p == 0), stop=(p == KP - 1))
                            xs = p2.tile([128, CH], F32, tag="xs", name="xs",
                                         bufs=3)
                            nc.vector.scalar_tensor_tensor(
                                xs[:], muf[:], csn_t[q][:], tout_ps[:],
                                op0=ALU.mult, op1=ALU.add)
                            nc.gpsimd.tensor_mul(xs[:], xs[:], rhof[:])
                            x1q = p2.tile([128, CH], F32, tag=f"x1_{q}",
                                          name=f"x1_{q}")
                            nc.vector.scalar_tensor_tensor(
                                x1q[:], xs[:], bp_t[q][:], xt2[q][:],
                                op0=ALU.add, op1=ALU.add)
                            x1b = p2.tile([128, CH], BF16, tag=f"x1b{q}",
                                          name=f"x1b{q}")
                            nc.scalar.activation(x1b[:], x1q[:], AF.Copy)
                            sq2 = p2.tile([128, CH], BF16, tag=f"sq2{q}",
                                          name=f"sq2{q}")
                            nc.scalar.activation(sq2[:], x1q[:], AF.Square)
                            x1l.append(x1q); x1bl.append(x1b); sq2l.append(sq2)
                            if q >= 1:
                                stats_mms(q - 1)
                        stats_mms(KP - 1)

                        negmu2 = p2.tile([128, CH], F32, tag="negmu2",
                                         name="negmu2", bufs=2)
                        nc.vector.tensor_scalar_mul(negmu2[:], ps_s2[:], -1.0)
                        var2 = p2.tile([128, CH], F32, tag="var2", name="var2",
                                       bufs=2)
                        nc.vector.tensor_mul(var2[:], negmu2[:], negmu2[:])
                        nc.vector.tensor_tensor(var2[:], ps_q2[:], var2[:],
                                                ALU.subtract)
                        nc.scalar.activation(var2[:], var2[:], AF.Sqrt,
                                             bias=eps_t[:])
                        rho2 = p2.tile([128, CH], F32, tag="rho2", name="rho2",
                                       bufs=2)
                        nc.vector.reciprocal(rho2[:], var2[:])

                        mg = []
                        for j in range(KP):
                            m1_ps = ps2.tile([128, CH], F32, tag="m1ps",
                                             name="m1_ps", bufs=2)
                            for p in range(KP):
                                nc.tensor.matmul(
                                    m1_ps[:],
                                    m1_t[p][:, j * 128 : (j + 1) * 128],
                                    x1bl[p][:],
                                    start=(p == 0), stop=(p == KP - 1))
                            ct = p2.tile([128, CH], F32, tag="ct", name="ct",
                                         bufs=3)
                            nc.vector.scalar_tensor_tensor(
                                ct[:], negmu2[:], rs1_t[j][:], m1_ps[:],
                                op0=ALU.mult, op1=ALU.add)
                            nc.vector.tensor_mul(ct[:], ct[:], rho2[:])
                            g = p2.tile([128, CH], BF16, tag=f"mg{j}",
                                        name=f"mg{j}")
                            nc.scalar.activation(g[:], ct[:], AF.Gelu,
                                                 bias=m1b_t[j][:])
                            mg.append(g)

                        mo_all = p2.tile([128, KP * CH], F32, tag="mo",
                                         name="mo")
                        for o in range(KP):
                            m2_ps = ps2.tile([128, CH], F32, tag="m2ps",
                                             name="m2_ps", bufs=2)
                            for j in range(KP):
                                nc.tensor.matmul(
                                    m2_ps[:],
                                    m2_t[j][:, o * 128 : (o + 1) * 128],
                                    mg[j][:],
                                    start=(j == 0), stop=(j == KP - 1))
                            nc.vector.scalar_tensor_tensor(
                                mo_all[:, o * CH : (o + 1) * CH], m2_ps[:],
                                m2b_t[o][:], x1l[o][:],
                                op0=ALU.add, op1=ALU.add)
                        nc.sync.dma_start(
                            blocked_dram_ap(outT_d, N, KP, 128, CH,
                                            col_off=c * CH),
                            mo_all[:])

            _loop_ctx.close()

    nc.compile()
    return nc


_CACHE = {}


def _get_nc(qk_bias, loop_R=1):
    key = (qk_bias, loop_R)
    if key not in _CACHE:
        _CACHE[key] = _build(qk_bias, loop_R)
    return _CACHE[key]


def _round_f32r(x):
    u = np.ascontiguousarray(x, np.float32).view(np.uint32)
    shift = 13
    bias = np.uint32((1 << (shift - 1)) - 1)
    lsb = (u >> np.uint32(shift)) & np.uint32(1)
    u2 = (u + bias + lsb) & np.uint32(~((1 << shift) - 1) & 0xFFFFFFFF)
    return u2.view(np.float32)


def kernel(**inputs):
    return _kernel_impl(inputs, loop_R=1)


def _kernel_impl(inputs, loop_R=1):
    x = np.ascontiguousarray(np.asarray(inputs["x"], np.float32))
    assert x.shape == (B, N, P), x.shape

    f32 = lambda k: np.asarray(inputs[k], np.float32)
    ln1_g, ln1_b = f32("ln1_g"), f32("ln1_b")
    ln2_g, ln2_b = f32("ln2_g"), f32("ln2_b")
    qk_w, v_w, v_b = f32("qk_w"), f32("v_w"), f32("v_b")
    t1_w, t1_b = f32("t1_w"), f32("t1_b")
    t2_w, t2_b = f32("t2_w"), f32("t2_b")
    m1_w, m1_b = f32("m1_w"), f32("m1_b")
    m2_w, m2_b = f32("m2_w"), f32("m2_b")

    qk_w_eff = qk_w * ln1_g[None, :]
    v_w_eff = v_w * ln1_g[None, :]
    v_b_eff = v_b + v_w @ ln1_b
    qk_b = qk_w @ ln1_b
    qk_bias = bool(np.any(qk_b != 0.0))
    m1_w_eff = m1_w * ln2_g[None, :]
    m1_b_eff = m1_b + m1_w @ ln2_b

    nc = _get_nc(qk_bias, loop_R)

    bf = ml_dtypes.bfloat16
    m1_bf = np.ascontiguousarray(m1_w_eff.T).astype(bf)
    m1_rs = m1_bf.astype(np.float32).sum(axis=0)
    base = {
        "qk_wT": _round_f32r(qk_w_eff.T),
        "vw": np.ascontiguousarray(v_w_eff).astype(bf),
        "t1_wT": _round_f32r(t1_w.T),
        "t2_wT": _round_f32r(t2_w.T),
        "m1_wT": m1_bf,
        "m2_wT": np.ascontiguousarray(m2_w.T).astype(bf),
        "v_b": np.ascontiguousarray(v_b_eff),
        "t1_b": np.ascontiguousarray(t1_b),
        "t2_b": np.ascontiguousarray(t2_b),
        "m1_b": np.ascontiguousarray(m1_b_eff),
        "m2_b": np.ascontiguousarray(m2_b),
        "m1_rs": np.ascontiguousarray(m1_rs),
        "nqcs": np.ascontiguousarray(-_round_f32r(qk_w_eff.T).sum(axis=0)),
        "ident": np.eye(128, dtype=np.float32),
    }
    if qk_bias:
        base["qk_b"] = np.ascontiguousarray(qk_b)

    in_maps = []
    for b in range(B):
        m = dict(base)
        m["xT"] = np.ascontiguousarray(x[b].T)
        in_maps.append(m)

    res = bass_utils.run_bass_kernel_spmd(nc, in_maps, core_ids=list(range(B)))
    out = np.empty((B, N, P), np.float32)
    for b in range(B):
        out[b] = res.results[b]["outT"].T
    return out
